# revision 11
# baseline (speedup 1.0000x reference)
"""BeatPooling segment-mean kernel for 8 Trainium2 NeuronCores.

Reference computation (per batch row):
    s = clip(bounds[:, 0], 0, T-1); e = max(s+1, min(bounds[:, 1], T))
    mean[m] = sum(frame[s_m:e_m]) / (e_m - s_m)            # via cumsum gather
    out = concat([mean, fourier(pos)], -1) @ W + b         # [M, D]

Sharding: data-parallel over B (one batch row per core). The Fourier/bias
term ff @ W[D:] + b is data-independent and folded on the host into a
[M, D] bias; clamp/count index arithmetic on the tiny bounds tensor is
also host-side.

Device pipeline per core (v2 — matmul cumsum, split gathers):
  1. stream the frame row as bf16 (8 MiB, 16 x 512 KiB DMAs)
  2. ONE bf16 PE matmul per [128t x 128d] tile against a constant
     upper-triangular ones matrix computes the block-local cumsum AND
     the [d, t] transpose in a single op (psum = frame_tile^T @ U)
  3. PSUM->SBUF drains rotate over DVE / ACT / GPSIMD into an
     interleaved gather table (two half-T windows, each with a leading
     zero t-slot; element (p, w, 1+t, c) = local cumsum of d-chunk c)
  4. per half-window, one gpsimd ap_gather fetches the 1024 segment
     boundary columns; out-of-window boundaries point at the zero slot
     so the two halves compose additively. Gather A runs at stream
     midpoint (hidden); only gather B is on the critical tail.
  5. cross-block prefixes are never added to the table: blocksum
     prefix scans P (tiny DVE scans) are folded into the projection via
     po += Ecomb^T @ (sum_c P_c^T @ W1_c) with a host-built {-1,0,+1}
     block-selection matrix Ecomb.
  6. seg^T = gather_e - gather_s (in place), projected with float32r
     matmuls (1 cycle/row at 512 moving cols); fused recip-scale + bias
     on the drain.
"""

import math

import numpy as np
import ml_dtypes

import concourse.bacc as bacc
import concourse.mybir as mybir
from concourse import bass_utils
from concourse.tile import TileContext

B, T, D, M = 8, 8192, 512, 512
POS_DIM = 32
P = 128
N_CORES = 8
TB = T // P            # 64 t-blocks
GROUPS = TB // 4       # 16 groups of 4 blocks (512 t each)
DC = D // P            # 4 d-chunks
MC = M // P            # 4 m-chunks
WT = T // 2            # 4096 t per gather window
WSZ = 1 + WT * DC      # window size in table elements (incl zero slot)

F32 = mybir.dt.float32
F32R = mybir.dt.float32r
BF16 = mybir.dt.bfloat16
I16 = mybir.dt.int16

_CACHED_NC = None


def _build_nc():
    nc = bacc.Bacc("TRN2", target_bir_lowering=False, debug=False,
                   num_devices=N_CORES)

    frame = nc.dram_tensor("frame", [GROUPS * P, 4 * D], BF16, kind="ExternalInput")
    u_in = nc.dram_tensor("u", [P, P], BF16, kind="ExternalInput")
    w1_in = nc.dram_tensor("w1", [D, D], F32R, kind="ExternalInput")
    bias_in = nc.dram_tensor("bias", [M, D], BF16, kind="ExternalInput")
    recip_in = nc.dram_tensor("recip", [P, MC], F32, kind="ExternalInput")
    idxa_in = nc.dram_tensor("idxa", [P, DC * 2 * M // 16], I16, kind="ExternalInput")
    idxb_in = nc.dram_tensor("idxb", [P, DC * 2 * M // 16], I16, kind="ExternalInput")
    ec_in = nc.dram_tensor("ecomb", [TB, M], F32, kind="ExternalInput")
    out = nc.dram_tensor("out", [M, D], F32, kind="ExternalOutput")

    add = mybir.AluOpType.add
    mult = mybir.AluOpType.mult
    bypass = mybir.AluOpType.bypass
    sub = mybir.AluOpType.subtract

    with TileContext(nc, num_cores=N_CORES) as tc:
        with (
            tc.tile_pool(name="const", bufs=1) as const,
            tc.tile_pool(name="staging", bufs=3) as staging,
            tc.tile_pool(name="psum", bufs=8, space="PSUM") as psum,
            tc.tile_pool(name="outp", bufs=2) as outp,
        ):
            # ---- long-lived tiles -------------------------------------
            u_t = const.tile([P, P], BF16, name="u")
            w1 = [const.tile([P, D], F32R, name=f"w1_{c}") for c in range(DC)]
            bias = [const.tile([P, D], BF16, name=f"bias_{m}") for m in range(MC)]
            recip = const.tile([P, MC], F32, name="recip")
            idxa = const.tile([P, DC * 2 * M // 16], I16, name="idxa")
            idxb = const.tile([P, DC * 2 * M // 16], I16, name="idxb")
            ec = const.tile([TB, M], F32, name="ec")
            # gather table: two half-T windows, each [zero-slot, 4096 t-slots]
            # interleaved over the 4 d-chunks
            table = const.tile([P, 2 * WSZ], F32, name="table")
            pfx = [const.tile([P, TB], F32, name=f"pfx_{c}") for c in range(DC)]
            qs = const.tile([TB, D], F32, name="qs")
            gout = const.tile([P, 2 * M * DC], F32, name="gout")
            gdiff_h = [const.tile([P, M * DC], F32R, name=f"gdiff_{h}")
                       for h in range(2)]


            nc.sync.dma_start(u_t[:], u_in.ap())
            nc.sync.dma_start(idxa[:], idxa_in.ap())
            nc.vector.memset(table[:, 0:1], 0.0)
            nc.vector.memset(table[:, WSZ:WSZ + 1], 0.0)

            po = [psum.tile([P, D], F32, name=f"po_{m}", tag="ps")
                  for m in range(MC)]

            # ---- stream frame: fused cumsum+transpose matmuls ----------
            # host pre-permutes so each partition's group line is 4 KiB
            # contiguous in HBM (full DMA line rate)
            drain_eng = [nc.vector, nc.scalar]
            for g in range(GROUPS):
                h = g // 8
                lt = (g - 8 * h) * 512
                stage = staging.tile([P, 4 * D], BF16, name="stage", tag="stage")
                nc.sync.dma_start(stage[:], frame.ap()[g * P:(g + 1) * P, :])
                for c in range(DC):
                    ps = psum.tile([P, 512], F32, name="ps", tag="ps")
                    for bb in range(4):
                        nc.tensor.matmul(
                            ps[:, bb * P:(bb + 1) * P],
                            lhsT=stage[:, bb * D + c * P: bb * D + (c + 1) * P],
                            rhs=u_t[:],
                            start=True,
                            stop=True,
                        )
                    eng = drain_eng[(g * DC + c) % 2]
                    # chunk-contiguous window layout: [zero, c0 t.., c1 t..]
                    dst = table[:, h * WSZ + 1 + c * WT + lt:
                                h * WSZ + 1 + c * WT + lt + 512]
                    if eng is nc.scalar:
                        nc.scalar.copy(dst, ps[:])
                    else:
                        eng.tensor_copy(dst, ps[:])

                # small-constant DMAs slotted between the big stage DMAs
                if g == 1:
                    nc.sync.dma_start(idxb[:], idxb_in.ap())
                    for c in range(DC):
                        nc.sync.dma_start(w1[c][:], w1_in.ap()[c * P:(c + 1) * P, :])
                if g == 9:
                    nc.sync.dma_start(ec[:], ec_in.ap())
                    for m in range(MC):
                        nc.sync.dma_start(bias[m][:], bias_in.ap()[m * P:(m + 1) * P, :])
                    nc.sync.dma_start(recip[:], recip_in.ap())

            # ---- tail: gathers, prefix scans, corrections, projection --
            # issued AFTER all stream work so the in-order engine FIFOs
            # never head-of-line block half B behind half A's tail.
            for h in range(2):
                idx_t = idxa if h == 0 else idxb
                nc.gpsimd.ap_gather(
                    gout[:],
                    table[:, h * WSZ:(h + 1) * WSZ],
                    idx_t[:],
                    channels=P,
                    num_elems=WSZ,
                    d=1,
                    num_idxs=DC * 2 * M,
                )
                for c in range(DC):
                    nc.vector.tensor_tensor_scan(
                        out=pfx[c][:, h * 32:(h + 1) * 32],
                        data0=table[:, h * WSZ + 1 + c * WT + P - 1: h * WSZ + 1 + (c + 1) * WT: P],
                        data1=table[:, 0:1].broadcast_to([P, 32]),
                        initial=(0.0 if h == 0 else pfx[c][:, 31:32]),
                        op0=add,
                        op1=bypass,
                    )
                gd = gdiff_h[h]
                nc.vector.tensor_tensor(
                    out=gd[:],
                    in0=gout[:, M * DC:2 * M * DC],
                    in1=gout[:, 0:M * DC],
                    op=sub,
                )
                if h == 0:
                    gva = gd[:].rearrange("p (i c) -> p i c", c=DC)
                    for m in range(MC):
                        for c in range(DC):
                            nc.tensor.matmul(
                                po[m][:],
                                lhsT=gva[:, m * P:(m + 1) * P, c],
                                rhs=w1[c][:],
                                start=(c == 0),
                                stop=False,
                            )
            # cross-block correction: po += Ecomb^T @ (sum_c P_c^T W1_c)
            qp = psum.tile([TB, D], F32, name="qp", tag="ps")
            for c in range(DC):
                nc.tensor.matmul(
                    qp[:],
                    lhsT=pfx[c][:],
                    rhs=w1[c][:].bitcast(F32),
                    start=(c == 0),
                    stop=(c == DC - 1),
                )
            nc.scalar.copy(qs[:], qp[:])
            for m in range(MC):
                nc.tensor.matmul(
                    po[m][:],
                    lhsT=ec[:, m * P:(m + 1) * P],
                    rhs=qs[:],
                    start=False,
                    stop=False,
                )
            gvb = gdiff_h[1][:].rearrange("p (i c) -> p i c", c=DC)
            for m in range(MC):
                for c in range(DC):
                    nc.tensor.matmul(
                        po[m][:],
                        lhsT=gvb[:, m * P:(m + 1) * P, c],
                        rhs=w1[c][:],
                        start=False,
                        stop=(c == DC - 1),
                    )

            # ---- out = recip * psum + bias ----------------------------
            for m in range(MC):
                ot = outp.tile([P, D], F32, name="ot", tag="ot")
                nc.vector.scalar_tensor_tensor(
                    out=ot[:],
                    in0=po[m][:],
                    scalar=recip[:, m:m + 1],
                    in1=bias[m][:],
                    op0=mult,
                    op1=add,
                )
                nc.sync.dma_start(out.ap()[m * P:(m + 1) * P, :], ot[:])

    nc.compile()
    return nc


def _fourier_features(pos, dim):
    half = dim // 2
    freqs = np.exp(np.linspace(0.0, math.log(1000.0), half))
    ang = pos[..., None] * freqs
    out = np.concatenate([np.sin(ang), np.cos(ang)], axis=-1)
    return out


def _wrap_idx(idx):
    """[N] int -> [128, N/16] int16 gather layout (16-partition wrap,
    replicated for the 8 gpsimd cores)."""
    n = idx.shape[0]
    wrapped = idx.reshape(n // 16, 16).T              # [16, N/16]
    return np.tile(wrapped, (8, 1)).astype(np.int16)  # [128, N/16]


def _host_prep(frame_emb, beat_bounds, W, b):
    """Per-core input maps (core i <- batch row i)."""
    s = np.clip(beat_bounds[:, :, 0], 0, T - 1).astype(np.int64)
    e = np.maximum(s + 1, np.minimum(beat_bounds[:, :, 1], T)).astype(np.int64)
    counts = (e - s).astype(np.float32)
    recip = (1.0 / counts).astype(np.float32)            # [B, M]

    pos = np.clip(np.arange(M, dtype=np.float64) / max(1, M - 1), 0.0, 1.0)
    ff = _fourier_features(pos, POS_DIM)                 # [M, 32]
    bias = (ff @ W[D:, :].astype(np.float64)
            + b.astype(np.float64)).astype(ml_dtypes.bfloat16)   # [M, D]
    w1 = np.ascontiguousarray(W[:D, :], dtype=np.float32)
    # pre-round to the fp32r bit pattern (11 explicit mantissa bits)
    w1 = (((w1.view(np.uint32) + 0x800) & np.uint32(0xFFFFF000))
          .view(np.float32))
    u = np.triu(np.ones((P, P), dtype=np.float32)).astype(ml_dtypes.bfloat16)

    in_maps = []
    for i in range(B):
        si = s[i] - 1                                    # -1 => zero slot
        ei = e[i] - 1
        idx_h = []
        for h in range(2):
            lo = WT * h
            sm = (si >= lo) & (si < lo + WT)
            em = (ei >= lo) & (ei < lo + WT)
            s_idx = np.where(sm, si - lo + 1, 0)      # [M], 0 = zero slot
            e_idx = np.where(em, ei - lo + 1, 0)
            flat = np.concatenate([s_idx, e_idx])     # [2M] slot-major
            # chunk-contiguous window: idx[i*4+c] = c*WT + local_t (+1)
            j = (flat[:, None] + np.where(flat[:, None] > 0,
                                          np.arange(DC)[None, :] * WT, 0))
            idx_h.append(_wrap_idx(j.reshape(-1)))
        # Ecomb[k, m]: +1 at blk(e-1)-1, -1 at blk(s-1)-1 (skip block 0 and
        # the s==0 sentinel); folds P_incl[k] into the projection
        ecomb = np.zeros((TB, M), dtype=np.float32)
        ke = ei // P - 1
        me = ke >= 0
        np.add.at(ecomb, (ke[me], np.arange(M)[me]), 1.0)
        ks = si // P - 1
        ms = (si >= 0) & (ks >= 0)
        np.subtract.at(ecomb, (ks[ms], np.arange(M)[ms]), 1.0)

        fr = frame_emb[i].astype(ml_dtypes.bfloat16)
        fr = np.ascontiguousarray(
            fr.reshape(GROUPS, 4, P, D).transpose(0, 2, 1, 3).reshape(
                GROUPS * P, 4 * D))
        in_maps.append({
            "frame": fr,
            "u": u,
            "w1": w1,
            "bias": bias,
            "recip": recip[i].reshape(MC, P).T.copy(),   # [P, MC]
            "idxa": idx_h[0],
            "idxb": idx_h[1],
            "ecomb": ecomb,
        })
    return in_maps


def get_nc():
    global _CACHED_NC
    if _CACHED_NC is None:
        _CACHED_NC = _build_nc()
    return _CACHED_NC


def kernel(frame_emb, beat_bounds, W, b, _trace=False):
    nc = get_nc()
    in_maps = _host_prep(np.asarray(frame_emb), np.asarray(beat_bounds),
                         np.asarray(W), np.asarray(b))
    res = bass_utils.run_bass_kernel_spmd(
        nc, in_maps, core_ids=list(range(N_CORES)), trace=_trace)
    out = np.stack([res.results[i]["out"] for i in range(B)], axis=0)
    if _trace:
        kernel.last_results = res
    return out


# revision 12
# speedup vs baseline: 2.3987x; 2.3987x over previous
"""BeatPooling segment-mean kernel for 8 Trainium2 NeuronCores.

Reference computation (per batch row):
    s = clip(bounds[:, 0], 0, T-1); e = max(s+1, min(bounds[:, 1], T))
    mean[m] = sum(frame[s_m:e_m]) / (e_m - s_m)            # via cumsum gather
    out = concat([mean, fourier(pos)], -1) @ W + b         # [M, D]

Sharding: data-parallel over B (one batch row per core). The Fourier/bias
term ff @ W[D:] + b is data-independent and folded on the host into a
[M, D] bias; clamp/count index arithmetic on the tiny bounds tensor is
also host-side.

Device pipeline per core (v2 — matmul cumsum, split gathers):
  1. stream the frame row as bf16 (8 MiB, 16 x 512 KiB DMAs)
  2. ONE bf16 PE matmul per [128t x 128d] tile against a constant
     upper-triangular ones matrix computes the block-local cumsum AND
     the [d, t] transpose in a single op (psum = frame_tile^T @ U)
  3. PSUM->SBUF drains rotate over DVE / ACT / GPSIMD into an
     interleaved gather table (two half-T windows, each with a leading
     zero t-slot; element (p, w, 1+t, c) = local cumsum of d-chunk c)
  4. per half-window, one gpsimd ap_gather fetches the 1024 segment
     boundary columns; out-of-window boundaries point at the zero slot
     so the two halves compose additively. Gather A runs at stream
     midpoint (hidden); only gather B is on the critical tail.
  5. cross-block prefixes are never added to the table: blocksum
     prefix scans P (tiny DVE scans) are folded into the projection via
     po += Ecomb^T @ (sum_c P_c^T @ W1_c) with a host-built {-1,0,+1}
     block-selection matrix Ecomb.
  6. seg^T = gather_e - gather_s (in place), projected with float32r
     matmuls (1 cycle/row at 512 moving cols); fused recip-scale + bias
     on the drain.
"""

import math

import numpy as np
import ml_dtypes

import concourse.bacc as bacc
import concourse.mybir as mybir
from concourse import bass_utils
from concourse.tile import TileContext

B, T, D, M = 8, 8192, 512, 512
POS_DIM = 32
P = 128
N_CORES = 8
TB = T // P            # 64 t-blocks
GROUPS = TB // 4       # 16 groups of 4 blocks (512 t each)
DC = D // P            # 4 d-chunks
MC = M // P            # 4 m-chunks
WT = T // 2            # 4096 t per gather window
WSZ = (WT + 1) * DC    # window size in table elements (incl zero t-slot)

F32 = mybir.dt.float32
F32R = mybir.dt.float32r
BF16 = mybir.dt.bfloat16
I16 = mybir.dt.int16

_CACHED_NC = None


def _build_nc():
    nc = bacc.Bacc("TRN2", target_bir_lowering=False, debug=False,
                   num_devices=N_CORES)

    frame = nc.dram_tensor("frame", [GROUPS * P, 4 * D], BF16, kind="ExternalInput")
    u_in = nc.dram_tensor("u", [P, P], BF16, kind="ExternalInput")
    w1_in = nc.dram_tensor("w1", [D, D], F32R, kind="ExternalInput")
    bias_in = nc.dram_tensor("bias", [M, D], BF16, kind="ExternalInput")
    recip_in = nc.dram_tensor("recip", [P, MC], F32, kind="ExternalInput")
    idxa_in = nc.dram_tensor("idxa", [P, 2 * M // 16], I16, kind="ExternalInput")
    idxb_in = nc.dram_tensor("idxb", [P, 2 * M // 16], I16, kind="ExternalInput")
    ec_in = nc.dram_tensor("ecomb", [TB, M], F32, kind="ExternalInput")
    out = nc.dram_tensor("out", [M, D], F32, kind="ExternalOutput")

    add = mybir.AluOpType.add
    mult = mybir.AluOpType.mult
    bypass = mybir.AluOpType.bypass
    sub = mybir.AluOpType.subtract

    with TileContext(nc, num_cores=N_CORES) as tc:
        with (
            tc.tile_pool(name="const", bufs=1) as const,
            tc.tile_pool(name="staging", bufs=3) as staging,
            tc.tile_pool(name="psum", bufs=8, space="PSUM") as psum,
            tc.tile_pool(name="outp", bufs=2) as outp,
        ):
            # ---- long-lived tiles -------------------------------------
            u_t = const.tile([P, P], BF16, name="u")
            w1 = [const.tile([P, D], F32R, name=f"w1_{c}") for c in range(DC)]
            bias = [const.tile([P, D], BF16, name=f"bias_{m}") for m in range(MC)]
            recip = const.tile([P, MC], F32, name="recip")
            idxa = const.tile([P, 2 * M // 16], I16, name="idxa")
            idxb = const.tile([P, 2 * M // 16], I16, name="idxb")
            ec = const.tile([TB, M], F32, name="ec")
            # gather table: two half-T windows, each [zero-slot, 4096 t-slots]
            # interleaved over the 4 d-chunks
            table = const.tile([P, 2 * WSZ], F32, name="table")
            pfx = [const.tile([P, TB], F32, name=f"pfx_{c}") for c in range(DC)]
            qs = const.tile([TB, D], F32, name="qs")
            gout = const.tile([P, 2 * M * DC], F32, name="gout")
            gdiff_h = [const.tile([P, M * DC], F32R, name=f"gdiff_{h}")
                       for h in range(2)]


            nc.sync.dma_start(u_t[:], u_in.ap())
            nc.sync.dma_start(idxa[:], idxa_in.ap())
            nc.vector.memset(table[:, 0:DC], 0.0)
            nc.vector.memset(table[:, WSZ:WSZ + DC], 0.0)

            po = [psum.tile([P, D], F32, name=f"po_{m}", tag="ps")
                  for m in range(MC)]

            # ---- stream frame: fused cumsum+transpose matmuls ----------
            # host pre-permutes so each partition's group line is 4 KiB
            # contiguous in HBM (full DMA line rate)
            drain_eng = [nc.vector, nc.scalar]
            for g in range(GROUPS):
                h = g // 8
                lt = (g - 8 * h) * 512
                stage = staging.tile([P, 4 * D], BF16, name="stage", tag="stage")
                nc.sync.dma_start(stage[:], frame.ap()[g * P:(g + 1) * P, :])
                for c in range(DC):
                    ps = psum.tile([P, 512], F32, name="ps", tag="ps")
                    for bb in range(4):
                        nc.tensor.matmul(
                            ps[:, bb * P:(bb + 1) * P],
                            lhsT=stage[:, bb * D + c * P: bb * D + (c + 1) * P],
                            rhs=u_t[:],
                            start=True,
                            stop=True,
                        )
                    eng = drain_eng[(g * DC + c) % 2]
                    # interleaved window layout: elem (1+t)*DC + c
                    dst = table[:, h * WSZ + (1 + lt) * DC + c:
                                h * WSZ + (1 + lt + 512) * DC:DC]
                    if eng is nc.scalar:
                        nc.scalar.copy(dst, ps[:])
                    else:
                        eng.tensor_copy(dst, ps[:])

                # small-constant DMAs slotted between the big stage DMAs
                if g == 1:
                    nc.sync.dma_start(idxb[:], idxb_in.ap())
                    for c in range(DC):
                        nc.sync.dma_start(w1[c][:], w1_in.ap()[c * P:(c + 1) * P, :])
                if g == 9:
                    nc.sync.dma_start(ec[:], ec_in.ap())
                    for m in range(MC):
                        nc.sync.dma_start(bias[m][:], bias_in.ap()[m * P:(m + 1) * P, :])
                    nc.sync.dma_start(recip[:], recip_in.ap())

            # ---- tail: gathers, prefix scans, corrections, projection --
            # issued AFTER all stream work so the in-order engine FIFOs
            # never head-of-line block half B behind half A's tail.
            for h in range(2):
                idx_t = idxa if h == 0 else idxb
                nc.gpsimd.ap_gather(
                    gout[:],
                    table[:, h * WSZ:(h + 1) * WSZ],
                    idx_t[:],
                    channels=P,
                    num_elems=WT + 1,
                    d=DC,
                    num_idxs=2 * M,
                )
                for c in range(DC):
                    nc.vector.tensor_tensor_scan(
                        out=pfx[c][:, h * 32:(h + 1) * 32],
                        data0=table[:, h * WSZ + P * DC + c: (h + 1) * WSZ: P * DC],
                        data1=table[:, 0:1].broadcast_to([P, 32]),
                        initial=(0.0 if h == 0 else pfx[c][:, 31:32]),
                        op0=add,
                        op1=bypass,
                    )
                gd = gdiff_h[h]
                nc.vector.tensor_tensor(
                    out=gd[:],
                    in0=gout[:, M * DC:2 * M * DC],
                    in1=gout[:, 0:M * DC],
                    op=sub,
                )
                if h == 0:
                    gva = gd[:].rearrange("p (i c) -> p i c", c=DC)
                    for m in range(MC):
                        for c in range(DC):
                            nc.tensor.matmul(
                                po[m][:],
                                lhsT=gva[:, m * P:(m + 1) * P, c],
                                rhs=w1[c][:],
                                start=(c == 0),
                                stop=False,
                            )
            # cross-block correction: po += Ecomb^T @ (sum_c P_c^T W1_c)
            qp = psum.tile([TB, D], F32, name="qp", tag="ps")
            for c in range(DC):
                nc.tensor.matmul(
                    qp[:],
                    lhsT=pfx[c][:],
                    rhs=w1[c][:].bitcast(F32),
                    start=(c == 0),
                    stop=(c == DC - 1),
                )
            nc.scalar.copy(qs[:], qp[:])
            for m in range(MC):
                nc.tensor.matmul(
                    po[m][:],
                    lhsT=ec[:, m * P:(m + 1) * P],
                    rhs=qs[:],
                    start=False,
                    stop=False,
                )
            gvb = gdiff_h[1][:].rearrange("p (i c) -> p i c", c=DC)
            for m in range(MC):
                for c in range(DC):
                    nc.tensor.matmul(
                        po[m][:],
                        lhsT=gvb[:, m * P:(m + 1) * P, c],
                        rhs=w1[c][:],
                        start=False,
                        stop=(c == DC - 1),
                    )

            # ---- out = recip * psum + bias ----------------------------
            for m in range(MC):
                ot = outp.tile([P, D], F32, name="ot", tag="ot")
                nc.vector.scalar_tensor_tensor(
                    out=ot[:],
                    in0=po[m][:],
                    scalar=recip[:, m:m + 1],
                    in1=bias[m][:],
                    op0=mult,
                    op1=add,
                )
                nc.sync.dma_start(out.ap()[m * P:(m + 1) * P, :], ot[:])

    nc.compile()
    return nc


def _fourier_features(pos, dim):
    half = dim // 2
    freqs = np.exp(np.linspace(0.0, math.log(1000.0), half))
    ang = pos[..., None] * freqs
    out = np.concatenate([np.sin(ang), np.cos(ang)], axis=-1)
    return out


def _wrap_idx(idx):
    """[N] int -> [128, N/16] int16 gather layout (16-partition wrap,
    replicated for the 8 gpsimd cores)."""
    n = idx.shape[0]
    wrapped = idx.reshape(n // 16, 16).T              # [16, N/16]
    return np.tile(wrapped, (8, 1)).astype(np.int16)  # [128, N/16]


def _host_prep(frame_emb, beat_bounds, W, b):
    """Per-core input maps (core i <- batch row i)."""
    s = np.clip(beat_bounds[:, :, 0], 0, T - 1).astype(np.int64)
    e = np.maximum(s + 1, np.minimum(beat_bounds[:, :, 1], T)).astype(np.int64)
    counts = (e - s).astype(np.float32)
    recip = (1.0 / counts).astype(np.float32)            # [B, M]

    pos = np.clip(np.arange(M, dtype=np.float64) / max(1, M - 1), 0.0, 1.0)
    ff = _fourier_features(pos, POS_DIM)                 # [M, 32]
    bias = (ff @ W[D:, :].astype(np.float64)
            + b.astype(np.float64)).astype(ml_dtypes.bfloat16)   # [M, D]
    w1 = np.ascontiguousarray(W[:D, :], dtype=np.float32)
    # pre-round to the fp32r bit pattern (11 explicit mantissa bits)
    w1 = (((w1.view(np.uint32) + 0x800) & np.uint32(0xFFFFF000))
          .view(np.float32))
    u = np.triu(np.ones((P, P), dtype=np.float32)).astype(ml_dtypes.bfloat16)

    in_maps = []
    for i in range(B):
        si = s[i] - 1                                    # -1 => zero slot
        ei = e[i] - 1
        idx_h = []
        for h in range(2):
            lo = WT * h
            sm = (si >= lo) & (si < lo + WT)
            em = (ei >= lo) & (ei < lo + WT)
            s_idx = np.where(sm, si - lo + 1, 0)      # [M], 0 = zero slot
            e_idx = np.where(em, ei - lo + 1, 0)
            idx_h.append(_wrap_idx(np.concatenate([s_idx, e_idx])))
        # Ecomb[k, m]: +1 at blk(e-1)-1, -1 at blk(s-1)-1 (skip block 0 and
        # the s==0 sentinel); folds P_incl[k] into the projection
        ecomb = np.zeros((TB, M), dtype=np.float32)
        ke = ei // P - 1
        me = ke >= 0
        np.add.at(ecomb, (ke[me], np.arange(M)[me]), 1.0)
        ks = si // P - 1
        ms = (si >= 0) & (ks >= 0)
        np.subtract.at(ecomb, (ks[ms], np.arange(M)[ms]), 1.0)

        fr = frame_emb[i].astype(ml_dtypes.bfloat16)
        fr = np.ascontiguousarray(
            fr.reshape(GROUPS, 4, P, D).transpose(0, 2, 1, 3).reshape(
                GROUPS * P, 4 * D))
        in_maps.append({
            "frame": fr,
            "u": u,
            "w1": w1,
            "bias": bias,
            "recip": recip[i].reshape(MC, P).T.copy(),   # [P, MC]
            "idxa": idx_h[0],
            "idxb": idx_h[1],
            "ecomb": ecomb,
        })
    return in_maps


def get_nc():
    global _CACHED_NC
    if _CACHED_NC is None:
        _CACHED_NC = _build_nc()
    return _CACHED_NC


def kernel(frame_emb, beat_bounds, W, b, _trace=False):
    nc = get_nc()
    in_maps = _host_prep(np.asarray(frame_emb), np.asarray(beat_bounds),
                         np.asarray(W), np.asarray(b))
    res = bass_utils.run_bass_kernel_spmd(
        nc, in_maps, core_ids=list(range(N_CORES)), trace=_trace)
    out = np.stack([res.results[i]["out"] for i in range(B)], axis=0)
    if _trace:
        kernel.last_results = res
    return out


# revision 24
# speedup vs baseline: 2.7924x; 1.1642x over previous
"""BeatPooling segment-mean kernel for 8 Trainium2 NeuronCores.

Reference computation (per batch row):
    s = clip(bounds[:, 0], 0, T-1); e = max(s+1, min(bounds[:, 1], T))
    mean[m] = sum(frame[s_m:e_m]) / (e_m - s_m)            # via cumsum gather
    out = concat([mean, fourier(pos)], -1) @ W + b         # [M, D]

Sharding: data-parallel over B (one batch row per core). The Fourier/bias
term ff @ W[D:] + b is data-independent and folded on the host into a
[M, D] bias; clamp/count index arithmetic on the tiny bounds tensor is
also host-side.

Device pipeline per core (v5 — native cumsum, SWDGE transpose-gather):
  1. stream the frame row as bf16 (8 MiB, 16 x 512 KiB DMAs, host
     pre-permuted so each partition line is 4 KiB contiguous)
  2. one bf16 PE matmul per 128-t block with a CONSTANT upper-triangular
     ones lhsT computes the block-local inclusive cumsum in native
     [t, d] orientation (psum = U^T @ frame_blk); constant weights avoid
     the per-tile LDWEIGHTS serialization of a transposing formulation
  3. drains split each fp32 PSUM row into bf16 hi + bf16 lo tables
     (hi on ACT, lo = psum - hi on DVE; hi+lo carries ~2^-16 relative
     precision); row t lives on partition t%128 at free offset
     (t//128)*1024B, plus one memset zero-rank at t=8192
  4. ONE pair of SWDGE dma_gathers (SBUF source, transpose mode) fetches
     all 1024 boundary rows + 64 block-end rows per table on the DMA
     engines (~2 descriptors/idx; the Q7 only generates descriptors).
     The 16-bit transpose writes tokens as [d-partition, chunk, slot] —
     exactly the lhsT layout the projection needs.
  5. blocksum prefix scans P over the gathered block-end rows fold the
     cross-block correction into the projection via a host-built
     {-1,0,+1} selection matrix: po += Ecomb^T @ (sum_c P_c^T @ W1_c)
     (this Q-chain runs in exact fp32; it is precision-critical)
  6. seg^T = (hi_e - hi_s) + (lo_e - lo_s) rounded to float32r, then 16
     f32r matmuls (1 cycle/row) project through W1; fused recip-scale +
     bias on the drain.
"""

import math

import numpy as np
import ml_dtypes

import concourse.bacc as bacc
import concourse.mybir as mybir
from concourse import bass_utils
from concourse.tile import TileContext

B, T, D, M = 8, 8192, 512, 512
POS_DIM = 32
P = 128
N_CORES = 8
TB = T // P            # 64 t-blocks
GROUPS = TB // 4       # 16 groups of 4 blocks (512 t each)
DC = D // P            # 4 d-chunks
MC = M // P            # 4 m-chunks
NRANK = TB + 1         # 64 cs ranks + 1 zero rank
ZIDX = T               # gather index of the zero row
GIDX = 640             # s-side idx count (SWDGE ring limit ~1k)
EIDX = 512             # e-side idx count (no pad needed)

F32 = mybir.dt.float32
F32R = mybir.dt.float32r
BF16 = mybir.dt.bfloat16
I16 = mybir.dt.int16

_CACHED_NC = None


def _build_nc():
    nc = bacc.Bacc("TRN2", target_bir_lowering=False, debug=False,
                   num_devices=N_CORES)

    frame = nc.dram_tensor("frame", [GROUPS * P, 4 * D], BF16,
                           kind="ExternalInput")
    u_in = nc.dram_tensor("u", [P, P], BF16, kind="ExternalInput")
    w1_in = nc.dram_tensor("w1", [D, D], F32R, kind="ExternalInput")
    bias_in = nc.dram_tensor("bias", [M, D], BF16, kind="ExternalInput")
    recip_in = nc.dram_tensor("recip", [P, MC], F32, kind="ExternalInput")
    idxs_in = nc.dram_tensor("idxs", [P, GIDX // 16], I16, kind="ExternalInput")
    idxe_in = nc.dram_tensor("idxe", [P, EIDX // 16], I16, kind="ExternalInput")
    ec_in = nc.dram_tensor("ecomb", [TB, M], F32, kind="ExternalInput")
    out = nc.dram_tensor("out", [M, D], F32, kind="ExternalOutput")

    add = mybir.AluOpType.add
    mult = mybir.AluOpType.mult
    bypass = mybir.AluOpType.bypass
    sub = mybir.AluOpType.subtract

    with TileContext(nc, num_cores=N_CORES) as tc:
        with (
            tc.tile_pool(name="const", bufs=1) as const,
            tc.tile_pool(name="staging", bufs=3) as staging,
            tc.tile_pool(name="psum", bufs=8, space="PSUM") as psum,
            tc.tile_pool(name="outp", bufs=2) as outp,
        ):
            # ---- long-lived tiles -------------------------------------
            u_t = const.tile([P, P], BF16, name="u")
            w1 = [const.tile([P, D], F32R, name=f"w1_{c}") for c in range(DC)]
            bias = [const.tile([P, D], BF16, name=f"bias_{m}") for m in range(MC)]
            recip = const.tile([P, MC], F32, name="recip")
            idxs_t = const.tile([P, GIDX // 16], I16, name="idxs")
            idxe_t = const.tile([P, EIDX // 16], I16, name="idxe")
            ec = const.tile([TB, M], F32, name="ec")
            # native-layout cumsum tables: row t on partition t%128 at
            # free offset (t//128)*512 elems; rank 64 is the zero row
            cs_hi = const.tile([P, NRANK * D], BF16, name="cs_hi")
            cs_lo = const.tile([P, NRANK * D], BF16, name="cs_lo")
            ghs = const.tile([P, DC * GIDX], BF16, name="ghs")
            ghe = const.tile([P, DC * EIDX], BF16, name="ghe")
            gls = const.tile([P, DC * GIDX], BF16, name="gls")
            gle = const.tile([P, DC * EIDX], BF16, name="gle")
            ssum = const.tile([P, DC * M], F32, name="ssum")
            besum = const.tile([P, DC * TB], F32, name="besum")
            pfx = [const.tile([P, TB], F32, name=f"pfx_{c}") for c in range(DC)]
            qs = const.tile([TB, D], F32, name="qs")
            gdiff = const.tile([P, DC * M], F32R, name="gdiff")
            scr = const.tile([P, DC * M], F32, name="scr")

            nc.sync.dma_start(u_t[:], u_in.ap())
            nc.sync.dma_start(idxs_t[:], idxs_in.ap())
            nc.sync.dma_start(idxe_t[:], idxe_in.ap())
            nc.vector.memset(cs_hi[:, TB * D:NRANK * D], 0.0)
            nc.vector.memset(cs_lo[:, TB * D:NRANK * D], 0.0)

            # ---- stream: constant-weight cumsum matmuls ----------------
            for g in range(GROUPS):
                stage = staging.tile([P, 4 * D], BF16, name="stage", tag="stage")
                nc.sync.dma_start(stage[:], frame.ap()[g * P:(g + 1) * P, :])
                for bb in range(4):
                    k = g * 4 + bb
                    ps = psum.tile([P, D], F32, name="ps", tag="ps")
                    nc.tensor.matmul(
                        ps[:],
                        lhsT=u_t[:],
                        rhs=stage[:, bb * D:(bb + 1) * D],
                        start=True,
                        stop=True,
                    )
                    hi = cs_hi[:, k * D:(k + 1) * D]
                    nc.scalar.copy(hi, ps[:])
                    nc.vector.tensor_tensor(
                        out=cs_lo[:, k * D:(k + 1) * D],
                        in0=ps[:],
                        in1=hi,
                        op=sub,
                    )
                if g == 1:
                    for c in range(DC):
                        nc.sync.dma_start(w1[c][:], w1_in.ap()[c * P:(c + 1) * P, :])
                if g == 9:
                    nc.sync.dma_start(ec[:], ec_in.ap())
                    for m in range(MC):
                        nc.sync.dma_start(bias[m][:], bias_in.ap()[m * P:(m + 1) * P, :])
                    nc.sync.dma_start(recip[:], recip_in.ap())

            # ---- tail: transpose-gathers on the DMA engines ------------
            # 4 gathers, <=640 idxs each (>1k idxs overflows the SWDGE
            # descriptor ring and wedges the exec unit); s-side first so
            # the fp32 Q-chain can start while the e-side still gathers
            for tbl, gout, idx_t, n in ((cs_hi, ghs, idxs_t, GIDX),
                                        (cs_lo, gls, idxs_t, GIDX),
                                        (cs_hi, ghe, idxe_t, EIDX),
                                        (cs_lo, gle, idxe_t, EIDX)):
                nc.gpsimd.dma_gather(
                    gout[:].rearrange("p (c i) -> p c i", c=DC),
                    tbl[:],
                    idx_t[:],
                    num_idxs=n,
                    num_idxs_reg=n,
                    elem_size=D,
                    transpose=True,
                    sbuf_tokens_per_rank=P,
                    sbuf_free_dim_per_rank=D * 2,
                )

            # blocksum prefix P: gathered block-end rows at slots
            # [2M, 2M+TB); combine hi+lo, then scan along k per chunk
            vhs = ghs[:].rearrange("p (c i) -> p c i", c=DC)
            vhe = ghe[:].rearrange("p (c i) -> p c i", c=DC)
            vls = gls[:].rearrange("p (c i) -> p c i", c=DC)
            vle = gle[:].rearrange("p (c i) -> p c i", c=DC)
            bev = besum[:].rearrange("p (c k) -> p c k", c=DC)
            nc.vector.tensor_tensor(
                out=bev[:, :, :],
                in0=vhs[:, :, M:M + TB],
                in1=vls[:, :, M:M + TB],
                op=add,
            )
            for c in range(DC):
                nc.vector.tensor_tensor_scan(
                    out=pfx[c][:],
                    data0=bev[:, c, :],
                    data1=recip[:, 0:1].broadcast_to([P, TB]),
                    initial=0.0,
                    op0=add,
                    op1=bypass,
                )
            # seg^T = (hi_e + lo_e) - (hi_s + lo_s), rounded to f32r;
            # the s-side sum runs while the e-side gathers are in flight
            gd = gdiff[:].rearrange("p (c i) -> p c i", c=DC)
            sc = scr[:].rearrange("p (c i) -> p c i", c=DC)
            sv = ssum[:].rearrange("p (c i) -> p c i", c=DC)
            nc.vector.tensor_tensor(
                out=sv[:, :, :],
                in0=vhs[:, :, 0:M],
                in1=vls[:, :, 0:M],
                op=add,
            )
            nc.vector.tensor_tensor(
                out=sc[:, :, :],
                in0=vhe[:, :, :],
                in1=vle[:, :, :],
                op=add,
            )
            nc.vector.tensor_tensor(
                out=gd[:, :, :],
                in0=sc[:, :, :],
                in1=sv[:, :, :],
                op=sub,
            )

            # cross-block correction (exact fp32): Q = sum_c P_c^T @ W1_c
            qp = psum.tile([TB, D], F32, name="qp", tag="ps")
            for c in range(DC):
                nc.tensor.matmul(
                    qp[:],
                    lhsT=pfx[c][:],
                    rhs=w1[c][:].bitcast(F32),
                    start=(c == 0),
                    stop=(c == DC - 1),
                )
            nc.scalar.copy(qs[:], qp[:])
            po = [psum.tile([P, D], F32, name=f"po_{m}", tag="ps")
                  for m in range(MC)]
            for m in range(MC):
                nc.tensor.matmul(
                    po[m][:],
                    lhsT=ec[:, m * P:(m + 1) * P],
                    rhs=qs[:],
                    start=True,
                    stop=False,
                )
            # projection of the segment sums (f32r, 1 cycle/row)
            for m in range(MC):
                for c in range(DC):
                    nc.tensor.matmul(
                        po[m][:],
                        lhsT=gd[:, c, m * P:(m + 1) * P],
                        rhs=w1[c][:],
                        start=False,
                        stop=(c == DC - 1),
                    )

            # ---- out = recip * psum + bias ----------------------------
            for m in range(MC):
                ot = outp.tile([P, D], F32, name="ot", tag="ot")
                nc.vector.scalar_tensor_tensor(
                    out=ot[:],
                    in0=po[m][:],
                    scalar=recip[:, m:m + 1],
                    in1=bias[m][:],
                    op0=mult,
                    op1=add,
                )
                nc.sync.dma_start(out.ap()[m * P:(m + 1) * P, :], ot[:])

    nc.compile()
    return nc


def _fourier_features(pos, dim):
    half = dim // 2
    freqs = np.exp(np.linspace(0.0, math.log(1000.0), half))
    ang = pos[..., None] * freqs
    out = np.concatenate([np.sin(ang), np.cos(ang)], axis=-1)
    return out


def _wrap_idx(idx):
    """[N] int -> [128, N/16] int16 gather layout (16-partition wrap,
    replicated for the 8 gpsimd cores)."""
    n = idx.shape[0]
    wrapped = idx.reshape(n // 16, 16).T              # [16, N/16]
    return np.tile(wrapped, (8, 1)).astype(np.int16)  # [128, N/16]


def _host_prep(frame_emb, beat_bounds, W, b):
    """Per-core input maps (core i <- batch row i)."""
    s = np.clip(beat_bounds[:, :, 0], 0, T - 1).astype(np.int64)
    e = np.maximum(s + 1, np.minimum(beat_bounds[:, :, 1], T)).astype(np.int64)
    counts = (e - s).astype(np.float32)
    recip = (1.0 / counts).astype(np.float32)            # [B, M]

    pos = np.clip(np.arange(M, dtype=np.float64) / max(1, M - 1), 0.0, 1.0)
    ff = _fourier_features(pos, POS_DIM)                 # [M, 32]
    bias = (ff @ W[D:, :].astype(np.float64)
            + b.astype(np.float64)).astype(ml_dtypes.bfloat16)   # [M, D]
    w1 = np.ascontiguousarray(W[:D, :], dtype=np.float32)
    # pre-round to the fp32r bit pattern (11 explicit mantissa bits)
    w1 = (((w1.view(np.uint32) + 0x800) & np.uint32(0xFFFFF000))
          .view(np.float32))
    u = np.triu(np.ones((P, P), dtype=np.float32)).astype(ml_dtypes.bfloat16)
    blockend = np.arange(TB, dtype=np.int64) * P + (P - 1)

    in_maps = []
    for i in range(B):
        si = s[i] - 1                                    # -1 => zero row
        ei = e[i] - 1
        s_idx = np.where(si >= 0, si, ZIDX)
        idx_s = np.concatenate([
            s_idx, blockend,
            np.full(GIDX - M - TB, ZIDX, dtype=np.int64),
        ])
        idx_e = ei
        # Ecomb[k, m]: +1 at blk(e-1)-1, -1 at blk(s-1)-1 (skip block 0 and
        # the s==0 sentinel); folds P_incl[k] into the projection
        ecomb = np.zeros((TB, M), dtype=np.float32)
        ke = ei // P - 1
        me = ke >= 0
        np.add.at(ecomb, (ke[me], np.arange(M)[me]), 1.0)
        ks = si // P - 1
        ms = (si >= 0) & (ks >= 0)
        np.subtract.at(ecomb, (ks[ms], np.arange(M)[ms]), 1.0)

        fr = frame_emb[i].astype(ml_dtypes.bfloat16)
        fr = np.ascontiguousarray(
            fr.reshape(GROUPS, 4, P, D).transpose(0, 2, 1, 3).reshape(
                GROUPS * P, 4 * D))
        in_maps.append({
            "frame": fr,
            "u": u,
            "w1": w1,
            "bias": bias,
            "recip": recip[i].reshape(MC, P).T.copy(),   # [P, MC]
            "idxs": _wrap_idx(idx_s),
            "idxe": _wrap_idx(idx_e),
            "ecomb": ecomb,
        })
    return in_maps


def get_nc():
    global _CACHED_NC
    if _CACHED_NC is None:
        _CACHED_NC = _build_nc()
    return _CACHED_NC


def kernel(frame_emb, beat_bounds, W, b, _trace=False):
    nc = get_nc()
    in_maps = _host_prep(np.asarray(frame_emb), np.asarray(beat_bounds),
                         np.asarray(W), np.asarray(b))
    res = bass_utils.run_bass_kernel_spmd(
        nc, in_maps, core_ids=list(range(N_CORES)), trace=_trace)
    out = np.stack([res.results[i]["out"] for i in range(B)], axis=0)
    if _trace:
        kernel.last_results = res
    return out


# revision 25
# speedup vs baseline: 2.8240x; 1.0113x over previous
"""BeatPooling segment-mean kernel for 8 Trainium2 NeuronCores.

Reference computation (per batch row):
    s = clip(bounds[:, 0], 0, T-1); e = max(s+1, min(bounds[:, 1], T))
    mean[m] = sum(frame[s_m:e_m]) / (e_m - s_m)            # via cumsum gather
    out = concat([mean, fourier(pos)], -1) @ W + b         # [M, D]

Sharding: data-parallel over B (one batch row per core). The Fourier/bias
term ff @ W[D:] + b is data-independent and folded on the host into a
[M, D] bias; clamp/count index arithmetic on the tiny bounds tensor is
also host-side.

Device pipeline per core (v5 — native cumsum, SWDGE transpose-gather):
  1. stream the frame row as bf16 (8 MiB, 16 x 512 KiB DMAs, host
     pre-permuted so each partition line is 4 KiB contiguous)
  2. one bf16 PE matmul per 128-t block with a CONSTANT upper-triangular
     ones lhsT computes the block-local inclusive cumsum in native
     [t, d] orientation (psum = U^T @ frame_blk); constant weights avoid
     the per-tile LDWEIGHTS serialization of a transposing formulation
  3. drains split each fp32 PSUM row into bf16 hi + bf16 lo tables
     (hi on ACT, lo = psum - hi on DVE; hi+lo carries ~2^-16 relative
     precision); row t lives on partition t%128 at free offset
     (t//128)*1024B, plus one memset zero-rank at t=8192
  4. ONE pair of SWDGE dma_gathers (SBUF source, transpose mode) fetches
     all 1024 boundary rows + 64 block-end rows per table on the DMA
     engines (~2 descriptors/idx; the Q7 only generates descriptors).
     The 16-bit transpose writes tokens as [d-partition, chunk, slot] —
     exactly the lhsT layout the projection needs.
  5. blocksum prefix scans P over the gathered block-end rows fold the
     cross-block correction into the projection via a host-built
     {-1,0,+1} selection matrix: po += Ecomb^T @ (sum_c P_c^T @ W1_c)
     (this Q-chain runs in exact fp32; it is precision-critical)
  6. seg^T = (hi_e - hi_s) + (lo_e - lo_s) rounded to float32r, then 16
     f32r matmuls (1 cycle/row) project through W1; fused recip-scale +
     bias on the drain.
"""

import math

import numpy as np
import ml_dtypes

import concourse.bacc as bacc
import concourse.mybir as mybir
from concourse import bass_utils
from concourse.tile import TileContext

B, T, D, M = 8, 8192, 512, 512
POS_DIM = 32
P = 128
N_CORES = 8
TB = T // P            # 64 t-blocks
GROUPS = TB // 4       # 16 groups of 4 blocks (512 t each)
DC = D // P            # 4 d-chunks
MC = M // P            # 4 m-chunks
NRANK = TB + 1         # 64 cs ranks + 1 zero rank
ZIDX = T               # gather index of the zero row
GIDX = 640             # s-side idx count (SWDGE ring limit ~1k)
EIDX = 512             # e-side idx count (no pad needed)

F32 = mybir.dt.float32
F32R = mybir.dt.float32r
BF16 = mybir.dt.bfloat16
I16 = mybir.dt.int16

_CACHED_NC = None


def _build_nc():
    nc = bacc.Bacc("TRN2", target_bir_lowering=False, debug=False,
                   num_devices=N_CORES)

    frame = nc.dram_tensor("frame", [GROUPS * P, 4 * D], BF16,
                           kind="ExternalInput")
    u_in = nc.dram_tensor("u", [P, P], BF16, kind="ExternalInput")
    w1_in = nc.dram_tensor("w1", [D, D], F32R, kind="ExternalInput")
    bias_in = nc.dram_tensor("bias", [M, D], BF16, kind="ExternalInput")
    recip_in = nc.dram_tensor("recip", [P, MC], F32, kind="ExternalInput")
    idxs_in = nc.dram_tensor("idxs", [P, GIDX // 16], I16, kind="ExternalInput")
    idxe_in = nc.dram_tensor("idxe", [P, EIDX // 16], I16, kind="ExternalInput")
    ec_in = nc.dram_tensor("ecomb", [TB, M], F32, kind="ExternalInput")
    out = nc.dram_tensor("out", [M, D], F32, kind="ExternalOutput")

    add = mybir.AluOpType.add
    mult = mybir.AluOpType.mult
    bypass = mybir.AluOpType.bypass
    sub = mybir.AluOpType.subtract

    with TileContext(nc, num_cores=N_CORES) as tc:
        with (
            tc.tile_pool(name="const", bufs=1) as const,
            tc.tile_pool(name="staging", bufs=3) as staging,
            tc.tile_pool(name="psum", bufs=8, space="PSUM") as psum,
            tc.tile_pool(name="outp", bufs=2) as outp,
        ):
            # ---- long-lived tiles -------------------------------------
            u_t = const.tile([P, P], BF16, name="u")
            w1 = [const.tile([P, D], F32R, name=f"w1_{c}") for c in range(DC)]
            bias = [const.tile([P, D], BF16, name=f"bias_{m}") for m in range(MC)]
            recip = const.tile([P, MC], F32, name="recip")
            idxs_t = const.tile([P, GIDX // 16], I16, name="idxs")
            idxe_t = const.tile([P, EIDX // 16], I16, name="idxe")
            ec = const.tile([TB, M], F32, name="ec")
            # native-layout cumsum tables: row t on partition t%128 at
            # free offset (t//128)*512 elems; rank 64 is the zero row
            cs_hi = const.tile([P, NRANK * D], BF16, name="cs_hi")
            cs_lo = const.tile([P, NRANK * D], BF16, name="cs_lo")
            ghs = const.tile([P, DC * GIDX], BF16, name="ghs")
            ghe = const.tile([P, DC * EIDX], BF16, name="ghe")
            gls = const.tile([P, DC * GIDX], BF16, name="gls")
            gle = const.tile([P, DC * EIDX], BF16, name="gle")
            ssum = const.tile([P, DC * M], F32, name="ssum")
            besum = const.tile([P, DC * TB], F32, name="besum")
            pfx = [const.tile([P, TB], F32, name=f"pfx_{c}") for c in range(DC)]
            qs = const.tile([TB, D], F32, name="qs")
            gdiff = const.tile([P, DC * M], F32R, name="gdiff")
            scr = const.tile([P, DC * M], F32, name="scr")

            nc.sync.dma_start(u_t[:], u_in.ap())
            nc.sync.dma_start(idxs_t[:], idxs_in.ap())
            nc.sync.dma_start(idxe_t[:], idxe_in.ap())
            nc.vector.memset(cs_hi[:, TB * D:NRANK * D], 0.0)
            nc.vector.memset(cs_lo[:, TB * D:NRANK * D], 0.0)

            po = [psum.tile([P, D], F32, name=f"po_{m}", tag="ps")
                  for m in range(MC)]

            # ---- stream: constant-weight cumsum matmuls ----------------
            for g in range(GROUPS):
                stage = staging.tile([P, 4 * D], BF16, name="stage", tag="stage")
                nc.sync.dma_start(stage[:], frame.ap()[g * P:(g + 1) * P, :])
                for bb in range(4):
                    k = g * 4 + bb
                    ps = psum.tile([P, D], F32, name="ps", tag="ps")
                    nc.tensor.matmul(
                        ps[:],
                        lhsT=u_t[:],
                        rhs=stage[:, bb * D:(bb + 1) * D],
                        start=True,
                        stop=True,
                    )
                    hi = cs_hi[:, k * D:(k + 1) * D]
                    nc.scalar.copy(hi, ps[:])
                    nc.vector.tensor_tensor(
                        out=cs_lo[:, k * D:(k + 1) * D],
                        in0=ps[:],
                        in1=hi,
                        op=sub,
                    )
                if g == 1:
                    for c in range(DC):
                        nc.sync.dma_start(w1[c][:], w1_in.ap()[c * P:(c + 1) * P, :])
                if g == 9:
                    nc.sync.dma_start(ec[:], ec_in.ap())
                    for m in range(MC):
                        nc.sync.dma_start(bias[m][:], bias_in.ap()[m * P:(m + 1) * P, :])
                    nc.sync.dma_start(recip[:], recip_in.ap())

            # ---- tail: transpose-gathers on the DMA engines ------------
            # 4 gathers, <=640 idxs each (>1k idxs overflows the SWDGE
            # descriptor ring and wedges the exec unit); s-side first so
            # the fp32 Q-chain can start while the e-side still gathers
            for tbl, gout, idx_t, n in ((cs_hi, ghs, idxs_t, GIDX),
                                        (cs_lo, gls, idxs_t, GIDX),
                                        (cs_hi, ghe, idxe_t, EIDX),
                                        (cs_lo, gle, idxe_t, EIDX)):
                nc.gpsimd.dma_gather(
                    gout[:].rearrange("p (c i) -> p c i", c=DC),
                    tbl[:],
                    idx_t[:],
                    num_idxs=n,
                    num_idxs_reg=n,
                    elem_size=D,
                    transpose=True,
                    sbuf_tokens_per_rank=P,
                    sbuf_free_dim_per_rank=D * 2,
                )

            # blocksum prefix P: gathered block-end rows at slots
            # [2M, 2M+TB); combine hi+lo, then scan along k per chunk
            vhs = ghs[:].rearrange("p (c i) -> p c i", c=DC)
            vhe = ghe[:].rearrange("p (c i) -> p c i", c=DC)
            vls = gls[:].rearrange("p (c i) -> p c i", c=DC)
            vle = gle[:].rearrange("p (c i) -> p c i", c=DC)
            bev = besum[:].rearrange("p (c k) -> p c k", c=DC)
            nc.vector.tensor_tensor(
                out=bev[:, :, :],
                in0=vhs[:, :, M:M + TB],
                in1=vls[:, :, M:M + TB],
                op=add,
            )
            for c in range(DC):
                nc.vector.tensor_tensor_scan(
                    out=pfx[c][:],
                    data0=bev[:, c, :],
                    data1=recip[:, 0:1].broadcast_to([P, TB]),
                    initial=0.0,
                    op0=add,
                    op1=bypass,
                )
            # seg^T = (hi_e + lo_e) - (hi_s + lo_s), rounded to f32r;
            # the s-side sum runs while the e-side gathers are in flight
            gd = gdiff[:].rearrange("p (c i) -> p c i", c=DC)
            sc = scr[:].rearrange("p (c i) -> p c i", c=DC)
            sv = ssum[:].rearrange("p (c i) -> p c i", c=DC)
            nc.vector.tensor_tensor(
                out=sv[:, :, :],
                in0=vhs[:, :, 0:M],
                in1=vls[:, :, 0:M],
                op=add,
            )
            nc.vector.tensor_tensor(
                out=sc[:, :, :],
                in0=vhe[:, :, :],
                in1=vle[:, :, :],
                op=add,
            )
            nc.vector.tensor_tensor(
                out=gd[:, :, :],
                in0=sc[:, :, :],
                in1=sv[:, :, :],
                op=sub,
            )

            # cross-block correction (exact fp32): Q = sum_c P_c^T @ W1_c
            qp = psum.tile([TB, D], F32, name="qp", tag="ps")
            for c in range(DC):
                nc.tensor.matmul(
                    qp[:],
                    lhsT=pfx[c][:],
                    rhs=w1[c][:].bitcast(F32),
                    start=(c == 0),
                    stop=(c == DC - 1),
                )
            nc.scalar.copy(qs[:], qp[:])
            for m in range(MC):
                nc.tensor.matmul(
                    po[m][:],
                    lhsT=ec[:, m * P:(m + 1) * P],
                    rhs=qs[:],
                    start=True,
                    stop=False,
                )
            # projection of the segment sums (f32r, 1 cycle/row)
            for m in range(MC):
                for c in range(DC):
                    nc.tensor.matmul(
                        po[m][:],
                        lhsT=gd[:, c, m * P:(m + 1) * P],
                        rhs=w1[c][:],
                        start=False,
                        stop=(c == DC - 1),
                    )

            # ---- out = recip * psum + bias ----------------------------
            for m in range(MC):
                ot = outp.tile([P, D], F32, name="ot", tag="ot")
                nc.vector.scalar_tensor_tensor(
                    out=ot[:],
                    in0=po[m][:],
                    scalar=recip[:, m:m + 1],
                    in1=bias[m][:],
                    op0=mult,
                    op1=add,
                )
                nc.sync.dma_start(out.ap()[m * P:(m + 1) * P, :], ot[:])

    nc.compile()
    return nc


def _fourier_features(pos, dim):
    half = dim // 2
    freqs = np.exp(np.linspace(0.0, math.log(1000.0), half))
    ang = pos[..., None] * freqs
    out = np.concatenate([np.sin(ang), np.cos(ang)], axis=-1)
    return out


def _wrap_idx(idx):
    """[N] int -> [128, N/16] int16 gather layout (16-partition wrap,
    replicated for the 8 gpsimd cores)."""
    n = idx.shape[0]
    wrapped = idx.reshape(n // 16, 16).T              # [16, N/16]
    return np.tile(wrapped, (8, 1)).astype(np.int16)  # [128, N/16]


def _host_prep(frame_emb, beat_bounds, W, b):
    """Per-core input maps (core i <- batch row i)."""
    s = np.clip(beat_bounds[:, :, 0], 0, T - 1).astype(np.int64)
    e = np.maximum(s + 1, np.minimum(beat_bounds[:, :, 1], T)).astype(np.int64)
    counts = (e - s).astype(np.float32)
    recip = (1.0 / counts).astype(np.float32)            # [B, M]

    pos = np.clip(np.arange(M, dtype=np.float64) / max(1, M - 1), 0.0, 1.0)
    ff = _fourier_features(pos, POS_DIM)                 # [M, 32]
    bias = (ff @ W[D:, :].astype(np.float64)
            + b.astype(np.float64)).astype(ml_dtypes.bfloat16)   # [M, D]
    w1 = np.ascontiguousarray(W[:D, :], dtype=np.float32)
    # pre-round to the fp32r bit pattern (11 explicit mantissa bits)
    w1 = (((w1.view(np.uint32) + 0x800) & np.uint32(0xFFFFF000))
          .view(np.float32))
    u = np.triu(np.ones((P, P), dtype=np.float32)).astype(ml_dtypes.bfloat16)
    blockend = np.arange(TB, dtype=np.int64) * P + (P - 1)

    in_maps = []
    for i in range(B):
        si = s[i] - 1                                    # -1 => zero row
        ei = e[i] - 1
        s_idx = np.where(si >= 0, si, ZIDX)
        idx_s = np.concatenate([
            s_idx, blockend,
            np.full(GIDX - M - TB, ZIDX, dtype=np.int64),
        ])
        idx_e = ei
        # Ecomb[k, m]: +1 at blk(e-1)-1, -1 at blk(s-1)-1 (skip block 0 and
        # the s==0 sentinel); folds P_incl[k] into the projection
        ecomb = np.zeros((TB, M), dtype=np.float32)
        ke = ei // P - 1
        me = ke >= 0
        np.add.at(ecomb, (ke[me], np.arange(M)[me]), 1.0)
        ks = si // P - 1
        ms = (si >= 0) & (ks >= 0)
        np.subtract.at(ecomb, (ks[ms], np.arange(M)[ms]), 1.0)

        fr = frame_emb[i].astype(ml_dtypes.bfloat16)
        fr = np.ascontiguousarray(
            fr.reshape(GROUPS, 4, P, D).transpose(0, 2, 1, 3).reshape(
                GROUPS * P, 4 * D))
        in_maps.append({
            "frame": fr,
            "u": u,
            "w1": w1,
            "bias": bias,
            "recip": recip[i].reshape(MC, P).T.copy(),   # [P, MC]
            "idxs": _wrap_idx(idx_s),
            "idxe": _wrap_idx(idx_e),
            "ecomb": ecomb,
        })
    return in_maps


def get_nc():
    global _CACHED_NC
    if _CACHED_NC is None:
        _CACHED_NC = _build_nc()
    return _CACHED_NC


def kernel(frame_emb, beat_bounds, W, b, _trace=False):
    nc = get_nc()
    in_maps = _host_prep(np.asarray(frame_emb), np.asarray(beat_bounds),
                         np.asarray(W), np.asarray(b))
    res = bass_utils.run_bass_kernel_spmd(
        nc, in_maps, core_ids=list(range(N_CORES)), trace=_trace)
    out = np.stack([res.results[i]["out"] for i in range(B)], axis=0)
    if _trace:
        kernel.last_results = res
    return out
